# revision 17
# baseline (speedup 1.0000x reference)
"""Trainium2 Bass kernel: per-pixel 5x5-patch channel covariance.

R[b,h,w,k,l] = (1/N) sum_n (p_kn - mu_k)(p_ln - mu_l)   (N=25, reflect pad)

Identity:  R = box5x5(S_k * S_l)/25 - mu_k * mu_l,  mu = box5x5(S)/25.
Separable box sums run as banded matmuls on TensorE; reflect padding is
folded into the band weights. Host pre-scales S by 1/5 so the two band
passes produce box/25 directly.

v3: pair products are computed ONCE on a 128-row tile (plus a folded
32-partition tile for the 4 tail rows, reshaped by DMA into the 4-row
lhsT the tail matmuls need), instead of twice on 68-row tiles. The H-box
runs as 3 matmuls per (channel, w-half): rows 0-63 from the head band,
rows 64-127 from the mid band, plus a 4-wide tail accumulation. Only the
136 upper-triangle pair channels are computed/DMA'd (pair-major); the
host mirrors to the full 16x16. Work is split across DVE / Act / GpSimd.

Sharding: 8 cores = 4 batches x 2 H-halves. Fully data parallel.
"""
import sys

sys.path.insert(0, "/opt/trn_rl_repo")

from contextlib import ExitStack

import numpy as np

import concourse.bacc as bacc
import concourse.mybir as mybir
import concourse.tile as tile
from concourse import bass_utils

B, K, H, W = 4, 16, 256, 256
HH = 128           # output rows per core
SR = 132           # shard rows (128 + 2 halo each side, edge-clamped)
NP = 136           # upper-triangle pairs, k-major: (0,0)..(0,15),(1,1)..
NCH = K + NP       # 152 channels: 0..15 mean, 16.. pairs (pr order)
NOCT = NCH // 8    # 19 channel octets (oct 0,1 = means; 2..18 = pairs)
NL0 = 72           # pairs in first L tile (octs 2..10); rest in second
F32 = mybir.dt.float32
BF16 = mybir.dt.bfloat16

# ---- tuning knobs (engine routing) ----
D_POOL_FRAC = 3    # every Nth sub unit routed Act-evac + GpSimd-sub
C_POOL_FRAC = 3    # every Nth M-octet built on GpSimd instead of DVE
B_DVE_FRAC = 4     # every Nth stage-1 evac copied by DVE instead of Act


def _reflect_idx(i, n):
    if i < 0:
        return -i
    if i >= n:
        return 2 * (n - 1) - i
    return i


def _build_bw():
    """[256 w'col, 256 wout] box weights with reflection folded; -> [128, 4*128]
    blocks indexed (oh, chunk): BW[:, (oh*2+c)*128 + wl] = M[c*128 + :, oh*128 + wl]."""
    M = np.zeros((W, W), dtype=np.float32)
    for w in range(W):
        for j in range(5):
            M[_reflect_idx(w - 2 + j, W), w] += 1.0
    out = np.zeros((128, 512), dtype=np.float32)
    for oh in range(2):
        for c in range(2):
            out[:, (oh * 2 + c) * 128:(oh * 2 + c) * 128 + 128] = \
                M[c * 128:(c + 1) * 128, oh * 128:(oh + 1) * 128]
    return out


def _build_brp(half):
    """H-box band, reflect folded, two partition-aligned packs:
    BR  [68, 68]: cols 0:64 head (out rows 0..63 from shard rows 0..67),
                  cols 64:68 tail (out rows 124..127 from shard rows 128..131,
                  band at partitions 0..3)
    BR2 [128, 64]: mid (out rows 64..127 from shard rows 64..127, band
                  stored at partitions 64..127)."""
    hbase = half * HH
    M132 = np.zeros((SR, HH), dtype=np.float32)
    for r in range(HH):
        for i in range(5):
            g = _reflect_idx(hbase + r - 2 + i, H)
            s = g - (hbase - 2)
            M132[s, r] += 1.0
    br = np.zeros((68, 68), dtype=np.float32)
    br[0:68, 0:64] = M132[0:68, 0:64]
    br[0:4, 64:68] = M132[128:132, 124:128]
    br2 = np.zeros((128, 64), dtype=np.float32)
    br2[64:128, :] = M132[64:128, 64:128]
    # coverage check: nothing outside the three packed blocks
    chk = M132.copy()
    chk[0:68, 0:64] = 0
    chk[64:128, 64:128] = 0
    chk[128:132, 124:128] = 0
    assert not chk.any(), "band pack dropped nonzero entries"
    return br, br2


def _ksegs_in_octet(oct_idx):
    """Pair channels live at ch 16..151 (pr k-major). For octet [oct*8, +8),
    return (j0, k, l0, nl): local offset j0, channel k, first l, count."""
    lo, hi = oct_idx * 8, oct_idx * 8 + 8
    segs = []
    p = 0
    for k in range(K):
        n = K - k
        s, e = 16 + p, 16 + p + n
        a, b = max(lo, s), min(hi, e)
        if a < b:
            segs.append((a - lo, k, k + (a - s), b - a))
        p += n
    return segs


def _pr0(k):
    """pr index of pair (k, k)."""
    return k * K - (k * (k - 1)) // 2


def _build_kernel():
    nc = bacc.Bacc("TRN2", target_bir_lowering=False, debug=False)
    SM_d = nc.dram_tensor("SM", [128, K, W], BF16, kind="ExternalInput").ap()
    ST_d = nc.dram_tensor("ST", [4, K, W], BF16, kind="ExternalInput").ap()
    SF_d = nc.dram_tensor("SF", [32, K, 32], BF16, kind="ExternalInput").ap()
    BR_d = nc.dram_tensor("BR", [68, 68], BF16, kind="ExternalInput").ap()
    BR2_d = nc.dram_tensor("BR2", [128, 64], BF16, kind="ExternalInput").ap()
    BW_d = nc.dram_tensor("BW", [128, 512], BF16, kind="ExternalInput").ap()
    # output: upper triangle only, [w, rt, pr, hh] bf16
    R_d = nc.dram_tensor("R", [W, 2, NP, 64], BF16, kind="ExternalOutput").ap()

    with tile.TileContext(nc) as tc, ExitStack() as ctx:
        const_p = ctx.enter_context(tc.tile_pool(name="const", bufs=1))
        sp_p = ctx.enter_context(tc.tile_pool(name="sp", bufs=1))
        tf_p = ctx.enter_context(tc.tile_pool(name="tf", bufs=1))
        L_p = ctx.enter_context(tc.tile_pool(name="L", bufs=1))
        t_p = ctx.enter_context(tc.tile_pool(name="tprod", bufs=3))
        i1_p = ctx.enter_context(tc.tile_pool(name="i1", bufs=4))
        mu_p = ctx.enter_context(tc.tile_pool(name="mu", bufs=1))
        m_p = ctx.enter_context(tc.tile_pool(name="mm", bufs=3))
        r_p = ctx.enter_context(tc.tile_pool(name="rout", bufs=1))
        e2_p = ctx.enter_context(tc.tile_pool(name="e2", bufs=2))
        ps1_p = ctx.enter_context(tc.tile_pool(name="ps1", bufs=2, space="PSUM"))
        ps2_p = ctx.enter_context(tc.tile_pool(name="ps2", bufs=4, space="PSUM"))

        brp = const_p.tile([68, 68], BF16)
        brp2 = const_p.tile([128, 64], BF16)
        bw = const_p.tile([128, 512], BF16)
        nc.sync.dma_start(brp[:], BR_d)
        nc.sync.dma_start(brp2[:], BR2_d)
        nc.sync.dma_start(bw[:], BW_d)

        sm = sp_p.tile([128, K, W], BF16)
        st = sp_p.tile([4, K, W], BF16)
        sf = sp_p.tile([32, K, 32], BF16)
        nc.sync.dma_start(sm[:], SM_d)
        nc.sync.dma_start(st[:], ST_d)
        nc.sync.dma_start(sf[:], SF_d)

        # ---- tail products (folded 32-partition tile), reshaped into L ----
        tf = tf_p.tile([32, NP, 32], BF16)
        for k in range(K):
            nl = K - k
            p0 = _pr0(k)
            in0 = sf[:, k, :].unsqueeze(1).broadcast_to([32, nl, 32])
            nc.vector.tensor_mul(tf[:, p0:p0 + nl, :], in0, sf[:, k:K, :])

        L = L_p.tile([4, NL0, W], BF16, name="L")
        for b in range(8):
            nc.sync.dma_start(L[:, :, b * 32:(b + 1) * 32],
                              tf[b * 4:(b + 1) * 4, 0:NL0, :])

        bra = brp[0:68, 0:64]
        brb = brp2[64:128, :]
        brc = brp[0:4, 64:68]
        bwh = [(bw[:, (oh * 2) * 128:(oh * 2) * 128 + 128],
                bw[:, (oh * 2 + 1) * 128:(oh * 2 + 1) * 128 + 128])
               for oh in range(2)]

        # mub_all[:, rt*2+oh, k, :]; rsb_all[:, rt, oh, pr, :]
        mub_all = mu_p.tile([128, 4, K, 64], BF16, name="mub")
        rsb_all = r_p.tile([128, 2, 2, NP, 64], BF16, name="rsb")

        def stage1(oc, L, L2):
            """products (pair octs) + H-box into a fresh per-octet i1 tile."""
            if oc < 2:   # mean channels read straight from sm / st
                T = sm[:, oc * 8:(oc + 1) * 8, :]
                TL = st[:, oc * 8:(oc + 1) * 8, :]
            else:
                Tt = t_p.tile([128, 8, W], BF16, name="T")
                for (j0, k, l0, nl) in _ksegs_in_octet(oc):
                    in0 = sm[:, k, :].unsqueeze(1).broadcast_to([128, nl, W])
                    nc.vector.tensor_mul(
                        Tt[:, j0:j0 + nl, :], in0, sm[:, l0:l0 + nl, :])
                T = Tt[:]
                pr = (oc - 2) * 8
                if pr < NL0:
                    TL = L[:, pr:pr + 8, :]
                else:
                    TL = L2[:, pr - NL0:pr - NL0 + 8, :]
            # i1oc layout: [w(128), rt(2), wchunk(2), ch(8), h(64)]
            i1oc = i1_p.tile([128, 2, 2, 8, 64], BF16, name="i1")
            for wh in range(2):
                ps1 = ps1_p.tile([128, 2, 8, 64], F32, name="ps1")
                ws = wh * 128
                for j in range(8):
                    nc.tensor.matmul(ps1[:, 0, j, :],
                                     T[0:68, j, ws:ws + 128], bra,
                                     start=True, stop=True)
                    nc.tensor.matmul(ps1[:, 1, j, :],
                                     T[64:128, j, ws:ws + 128], brb,
                                     start=True, stop=False,
                                     skip_group_check=True)
                    nc.tensor.matmul(ps1[:, 1, j, 60:64],
                                     TL[0:4, j, ws:ws + 128], brc,
                                     start=False, stop=True,
                                     skip_group_check=True)
                eng = nc.vector if (2 * oc + wh) % B_DVE_FRAC == 1 \
                    else nc.scalar
                eng_copy = (nc.vector.tensor_copy if eng is nc.vector
                            else nc.scalar.copy)
                eng_copy(i1oc[:, :, wh], ps1[:])
            return i1oc

        def p2(i1oc, oc, rt, oh):
            (bwa, bwb) = bwh[oh]
            ps2 = ps2_p.tile([128, 8, 64], F32, name="ps2")
            nc.tensor.matmul(ps2[:], bwa, i1oc[:, rt, 0],
                             start=True, stop=False)
            nc.tensor.matmul(ps2[:], bwb, i1oc[:, rt, 1],
                             start=False, stop=True)
            return ps2

        # ---- mean octets -> mub ----
        i1m = [stage1(oc, None, None) for oc in range(2)]
        for rt in range(2):
            for oh in range(2):
                for mo in range(2):
                    psm = p2(i1m[mo], mo, rt, oh)
                    nc.scalar.mul(mub_all[:, rt * 2 + oh, mo * 8:(mo + 1) * 8],
                                  psm[:], 0.2)

        # ---- pair octets: stage1 + stage2 interleaved ----
        L2 = None
        for oc in range(2, NOCT):
            i1oc = stage1(oc, L, L2)
            if oc == 10:   # L fully consumed; refill with remaining pairs
                L2 = L_p.tile([4, NL0, W], BF16, name="L")
                for b in range(8):
                    nc.sync.dma_start(L2[:, 0:NP - NL0, b * 32:(b + 1) * 32],
                                      tf[b * 4:(b + 1) * 4, NL0:NP, :])
            pr = (oc - 2) * 8
            for rt in range(2):
                for oh in range(2):
                    ps2 = p2(i1oc, oc, rt, oh)
                    mub = mub_all[:, rt * 2 + oh]
                    gunit = (oc - 2) * 4 + rt * 2 + oh
                    # M(oct) = mu_k * mu_l, built just in time
                    M = m_p.tile([128, 8, 64], BF16, name="M")
                    meng = nc.gpsimd if gunit % C_POOL_FRAC == 1 else nc.vector
                    for (j0, k, l0, nl) in _ksegs_in_octet(oc):
                        in0 = mub[:, k, :].unsqueeze(1).broadcast_to(
                            [128, nl, 64])
                        meng.tensor_mul(M[:, j0:j0 + nl, :], in0,
                                        mub[:, l0:l0 + nl, :])
                    dst = rsb_all[:, rt, oh, pr:pr + 8, :]
                    if gunit % D_POOL_FRAC == D_POOL_FRAC - 1:
                        e2 = e2_p.tile([128, 8, 64], BF16, name="e2")
                        nc.scalar.copy(e2[:], ps2[:])
                        nc.gpsimd.tensor_sub(dst, e2[:], M[:])
                    else:
                        nc.vector.tensor_sub(dst, ps2[:], M[:])

        for rt in range(2):
            for oh in range(2):
                nc.sync.dma_start(R_d[oh * 128:(oh + 1) * 128, rt],
                                  rsb_all[:, rt, oh])

    nc.compile()
    return nc


_NC_CACHE = {}


def _get_nc():
    if "nc" not in _NC_CACHE:
        _NC_CACHE["nc"] = _build_kernel()
    return _NC_CACHE["nc"]


def _prep_in_maps(S):
    S = np.asarray(S, dtype=np.float32)
    np_bf16 = mybir.dt.np(BF16)
    bw = _build_bw().astype(np_bf16)
    brs = [tuple(x.astype(np_bf16) for x in _build_brp(h)) for h in range(2)]
    Ss = S * np.float32(0.2)
    in_maps = []
    for b in range(B):
        for half in range(2):
            hbase = half * HH
            rows = np.clip(np.arange(hbase - 2, hbase + 130), 0, H - 1)
            shard = Ss[b][:, rows, :].transpose(1, 0, 2)   # [132, K, 256]
            shard = np.ascontiguousarray(shard).astype(np_bf16)
            sm = shard[0:128]
            stail = np.ascontiguousarray(shard[128:132])   # [4, K, 256]
            # fold: SF[wdiv*4 + r', c, j] = ST[r', c, wdiv*32 + j]
            sfold = np.ascontiguousarray(
                stail.reshape(4, K, 8, 32).transpose(2, 0, 1, 3)
                .reshape(32, K, 32))
            in_maps.append({"SM": sm, "ST": stail, "SF": sfold,
                            "BR": brs[half][0], "BR2": brs[half][1],
                            "BW": bw})
    return in_maps


# upper-tri gather index: IU[k, l] = pr for (min,max)
_IU = np.zeros((K, K), dtype=np.int64)
for _k in range(K):
    for _l in range(_k, K):
        _IU[_k, _l] = _IU[_l, _k] = _pr0(_k) + (_l - _k)


def _assemble(results):
    out = np.empty((B, H, W, K, K), dtype=np.float32)
    for i in range(8):
        b, half = divmod(i, 2)
        rd = np.asarray(results[i]["R"]).astype(np.float32)  # [256, 2, 136, 64]
        tri = rd.transpose(1, 3, 0, 2).reshape(HH, W, NP)    # [h, w, pr]
        out[b, half * HH:(half + 1) * HH] = tri[:, :, _IU]
    return out


def kernel(S):
    """S: [4, 16, 256, 256] float32 -> R: [4, 256, 256, 16, 16] float32."""
    nc = _get_nc()
    in_maps = _prep_in_maps(S)
    res = bass_utils.run_bass_kernel_spmd(nc, in_maps, list(range(8)))
    return _assemble(res.results)


# revision 21
# speedup vs baseline: 1.1209x; 1.1209x over previous
"""Trainium2 Bass kernel: per-pixel 5x5-patch channel covariance.

R[b,h,w,k,l] = (1/N) sum_n (p_kn - mu_k)(p_ln - mu_l)   (N=25, reflect pad)

Identity:  R = box5x5(S_k * S_l)/25 - mu_k * mu_l,  mu = box5x5(S)/25.
Separable box sums run as banded matmuls on TensorE; reflect padding is
folded into the band weights. Host pre-scales S by 1/5 so the two band
passes produce box/25 directly.

v3: pair products are computed ONCE on a 128-row tile (plus a folded
32-partition tile for the 4 tail rows, reshaped by DMA into the 4-row
lhsT the tail matmuls need), instead of twice on 68-row tiles. The H-box
runs as 3 matmuls per (channel, w-half): rows 0-63 from the head band,
rows 64-127 from the mid band, plus a 4-wide tail accumulation. Only the
136 upper-triangle pair channels are computed/DMA'd (pair-major); the
host mirrors to the full 16x16. Work is split across DVE / Act / GpSimd.

Sharding: 8 cores = 4 batches x 2 H-halves. Fully data parallel.
"""
import sys

sys.path.insert(0, "/opt/trn_rl_repo")

from contextlib import ExitStack

import numpy as np

import concourse.bacc as bacc
import concourse.mybir as mybir
import concourse.tile as tile
from concourse import bass_utils

B, K, H, W = 4, 16, 256, 256
HH = 128           # output rows per core
SR = 132           # shard rows (128 + 2 halo each side, edge-clamped)
NP = 136           # upper-triangle pairs, k-major: (0,0)..(0,15),(1,1)..
NCH = K + NP       # 152 channels: 0..15 mean, 16.. pairs (pr order)
NOCT = NCH // 8    # 19 channel octets (oct 0,1 = means; 2..18 = pairs)
NL0 = 72           # pairs in first L tile (octs 2..10); rest in second
F32 = mybir.dt.float32
BF16 = mybir.dt.bfloat16

# ---- tuning knobs (engine routing) ----
D_POOL_FRAC = 3    # every Nth sub unit routed Act-evac + GpSimd-sub
C_POOL_FRAC = 3    # every Nth M-octet built on GpSimd instead of DVE
B_DVE_FRAC = 999   # every Nth stage-1 evac copied by DVE instead of Act


def _reflect_idx(i, n):
    if i < 0:
        return -i
    if i >= n:
        return 2 * (n - 1) - i
    return i


def _build_bw():
    """[256 w'col, 256 wout] box weights with reflection folded; -> [128, 4*128]
    blocks indexed (oh, chunk): BW[:, (oh*2+c)*128 + wl] = M[c*128 + :, oh*128 + wl]."""
    M = np.zeros((W, W), dtype=np.float32)
    for w in range(W):
        for j in range(5):
            M[_reflect_idx(w - 2 + j, W), w] += 1.0
    out = np.zeros((128, 512), dtype=np.float32)
    for oh in range(2):
        for c in range(2):
            out[:, (oh * 2 + c) * 128:(oh * 2 + c) * 128 + 128] = \
                M[c * 128:(c + 1) * 128, oh * 128:(oh + 1) * 128]
    return out


def _build_brp(half):
    """H-box band, reflect folded, two partition-aligned packs:
    BR  [68, 68]: cols 0:64 head (out rows 0..63 from shard rows 0..67),
                  cols 64:68 tail (out rows 124..127 from shard rows 128..131,
                  band at partitions 0..3)
    BR2 [128, 64]: mid (out rows 64..127 from shard rows 64..127, band
                  stored at partitions 64..127)."""
    hbase = half * HH
    M132 = np.zeros((SR, HH), dtype=np.float32)
    for r in range(HH):
        for i in range(5):
            g = _reflect_idx(hbase + r - 2 + i, H)
            s = g - (hbase - 2)
            M132[s, r] += 1.0
    br = np.zeros((68, 68), dtype=np.float32)
    br[0:68, 0:64] = M132[0:68, 0:64]
    br[0:4, 64:68] = M132[128:132, 124:128]
    br2 = np.zeros((128, 64), dtype=np.float32)
    br2[64:128, :] = M132[64:128, 64:128]
    # coverage check: nothing outside the three packed blocks
    chk = M132.copy()
    chk[0:68, 0:64] = 0
    chk[64:128, 64:128] = 0
    chk[128:132, 124:128] = 0
    assert not chk.any(), "band pack dropped nonzero entries"
    return br, br2


def _ksegs_in_octet(oct_idx):
    """Pair channels live at ch 16..151 (pr k-major). For octet [oct*8, +8),
    return (j0, k, l0, nl): local offset j0, channel k, first l, count."""
    lo, hi = oct_idx * 8, oct_idx * 8 + 8
    segs = []
    p = 0
    for k in range(K):
        n = K - k
        s, e = 16 + p, 16 + p + n
        a, b = max(lo, s), min(hi, e)
        if a < b:
            segs.append((a - lo, k, k + (a - s), b - a))
        p += n
    return segs


def _pr0(k):
    """pr index of pair (k, k)."""
    return k * K - (k * (k - 1)) // 2


def _build_kernel():
    nc = bacc.Bacc("TRN2", target_bir_lowering=False, debug=False)
    SM_d = nc.dram_tensor("SM", [128, K, W], BF16, kind="ExternalInput").ap()
    ST_d = nc.dram_tensor("ST", [4, K, W], BF16, kind="ExternalInput").ap()
    SF_d = nc.dram_tensor("SF", [32, K, 32], BF16, kind="ExternalInput").ap()
    BR_d = nc.dram_tensor("BR", [68, 68], BF16, kind="ExternalInput").ap()
    BR2_d = nc.dram_tensor("BR2", [128, 64], BF16, kind="ExternalInput").ap()
    BW_d = nc.dram_tensor("BW", [128, 512], BF16, kind="ExternalInput").ap()
    # output: upper triangle only, [w, rt, pr, hh] bf16
    R_d = nc.dram_tensor("R", [W, 2, NP, 64], BF16, kind="ExternalOutput").ap()

    with tile.TileContext(nc) as tc, ExitStack() as ctx:
        const_p = ctx.enter_context(tc.tile_pool(name="const", bufs=1))
        sp_p = ctx.enter_context(tc.tile_pool(name="sp", bufs=1))
        tf_p = ctx.enter_context(tc.tile_pool(name="tf", bufs=1))
        L_p = ctx.enter_context(tc.tile_pool(name="L", bufs=1))
        t_p = ctx.enter_context(tc.tile_pool(name="tprod", bufs=3))
        i1_p = ctx.enter_context(tc.tile_pool(name="i1", bufs=4))
        mu_p = ctx.enter_context(tc.tile_pool(name="mu", bufs=1))
        m_p = ctx.enter_context(tc.tile_pool(name="mm", bufs=3))
        r_p = ctx.enter_context(tc.tile_pool(name="rout", bufs=1))
        e2_p = ctx.enter_context(tc.tile_pool(name="e2", bufs=2))
        ps1_p = ctx.enter_context(tc.tile_pool(name="ps1", bufs=2, space="PSUM"))
        ps2_p = ctx.enter_context(tc.tile_pool(name="ps2", bufs=4, space="PSUM"))

        brp = const_p.tile([68, 68], BF16)
        brp2 = const_p.tile([128, 64], BF16)
        bw = const_p.tile([128, 512], BF16)
        nc.sync.dma_start(brp[:], BR_d)
        nc.sync.dma_start(brp2[:], BR2_d)
        nc.sync.dma_start(bw[:], BW_d)

        sm = sp_p.tile([128, K, W], BF16)
        st = sp_p.tile([4, K, W], BF16)
        sf = sp_p.tile([32, K, 32], BF16)
        nc.sync.dma_start(sm[:], SM_d)
        nc.sync.dma_start(st[:], ST_d)
        nc.sync.dma_start(sf[:], SF_d)

        # ---- tail products (folded 32-partition tile), reshaped into L ----
        tf = tf_p.tile([32, NP, 32], BF16)
        for k in range(K):
            nl = K - k
            p0 = _pr0(k)
            in0 = sf[:, k, :].unsqueeze(1).broadcast_to([32, nl, 32])
            nc.vector.tensor_mul(tf[:, p0:p0 + nl, :], in0, sf[:, k:K, :])

        L = L_p.tile([4, NL0, W], BF16, name="L")
        for b in range(8):
            nc.sync.dma_start(L[:, :, b * 32:(b + 1) * 32],
                              tf[b * 4:(b + 1) * 4, 0:NL0, :])

        bra = brp[0:68, 0:64]
        brb = brp2[64:128, :]
        brc = brp[0:4, 64:68]
        bwh = [(bw[:, (oh * 2) * 128:(oh * 2) * 128 + 128],
                bw[:, (oh * 2 + 1) * 128:(oh * 2 + 1) * 128 + 128])
               for oh in range(2)]

        # mub_all[:, rt*2+oh, k, :]; rsb_all[:, rt, oh, pr, :]
        mub_all = mu_p.tile([128, 4, K, 64], BF16, name="mub")
        rsb_all = r_p.tile([128, 2, 2, NP, 64], BF16, name="rsb")

        pending_evacs = []

        def stage1(oc, L, L2, defer_dve_evac=True):
            """products (pair octs) + H-box into a fresh per-octet i1 tile.
            DVE-routed psum evacs are deferred to the next iteration via
            pending_evacs to keep the in-order DVE stream stall-free."""
            if oc < 2:   # mean channels read straight from sm / st
                T = sm[:, oc * 8:(oc + 1) * 8, :]
                TL = st[:, oc * 8:(oc + 1) * 8, :]
            else:
                Tt = t_p.tile([128, 8, W], BF16, name="T")
                for (j0, k, l0, nl) in _ksegs_in_octet(oc):
                    in0 = sm[:, k, :].unsqueeze(1).broadcast_to([128, nl, W])
                    nc.vector.tensor_mul(
                        Tt[:, j0:j0 + nl, :], in0, sm[:, l0:l0 + nl, :])
                T = Tt[:]
                pr = (oc - 2) * 8
                if pr < NL0:
                    TL = L[:, pr:pr + 8, :]
                else:
                    TL = L2[:, pr - NL0:pr - NL0 + 8, :]
            # i1oc layout: [w(128), rt(2), wchunk(2), ch(8), h(64)]
            i1oc = i1_p.tile([128, 2, 2, 8, 64], BF16, name="i1")
            for wh in range(2):
                ps1 = ps1_p.tile([128, 2, 8, 64], F32, name="ps1")
                ws = wh * 128
                for j in range(8):
                    nc.tensor.matmul(ps1[:, 0, j, :],
                                     T[0:68, j, ws:ws + 128], bra,
                                     start=True, stop=True)
                    nc.tensor.matmul(ps1[:, 1, j, :],
                                     T[64:128, j, ws:ws + 128], brb,
                                     start=True, stop=False,
                                     skip_group_check=True)
                    nc.tensor.matmul(ps1[:, 1, j, 60:64],
                                     TL[0:4, j, ws:ws + 128], brc,
                                     start=False, stop=True,
                                     skip_group_check=True)
                if (defer_dve_evac and oc >= 2
                        and (2 * oc + wh) % B_DVE_FRAC == 1):
                    pending_evacs.append((i1oc, wh, ps1))
                else:
                    nc.scalar.copy(i1oc[:, :, wh], ps1[:])
            return i1oc

        def flush_evacs():
            while pending_evacs:
                i1oc, wh, ps1 = pending_evacs.pop(0)
                nc.vector.tensor_copy(i1oc[:, :, wh], ps1[:])

        def p2(i1oc, oc, rt, oh):
            (bwa, bwb) = bwh[oh]
            ps2 = ps2_p.tile([128, 8, 64], F32, name="ps2")
            nc.tensor.matmul(ps2[:], bwa, i1oc[:, rt, 0],
                             start=True, stop=False)
            nc.tensor.matmul(ps2[:], bwb, i1oc[:, rt, 1],
                             start=False, stop=True)
            return ps2

        # ---- mean octets -> mub ----
        i1m = [stage1(oc, None, None) for oc in range(2)]
        for rt in range(2):
            for oh in range(2):
                for mo in range(2):
                    psm = p2(i1m[mo], mo, rt, oh)
                    nc.scalar.mul(mub_all[:, rt * 2 + oh, mo * 8:(mo + 1) * 8],
                                  psm[:], 0.2)

        def stage2(i1oc, oc):
            pr = (oc - 2) * 8
            units = []
            for rt in range(2):
                for oh in range(2):
                    units.append((p2(i1oc, oc, rt, oh), rt, oh))
            for ps2, rt, oh in units:
                mub = mub_all[:, rt * 2 + oh]
                gunit = (oc - 2) * 4 + rt * 2 + oh
                # M(oct) = mu_k * mu_l, built just in time
                M = m_p.tile([128, 8, 64], BF16, name="M")
                meng = nc.gpsimd if gunit % C_POOL_FRAC == 1 else nc.vector
                for (j0, k, l0, nl) in _ksegs_in_octet(oc):
                    in0 = mub[:, k, :].unsqueeze(1).broadcast_to(
                        [128, nl, 64])
                    meng.tensor_mul(M[:, j0:j0 + nl, :], in0,
                                    mub[:, l0:l0 + nl, :])
                dst = rsb_all[:, rt, oh, pr:pr + 8, :]
                if gunit % D_POOL_FRAC == D_POOL_FRAC - 1:
                    e2 = e2_p.tile([128, 8, 64], BF16, name="e2")
                    nc.scalar.copy(e2[:], ps2[:])
                    nc.gpsimd.tensor_sub(dst, e2[:], M[:])
                else:
                    nc.vector.tensor_sub(dst, ps2[:], M[:])

        # ---- pair octets: software pipelined, stage2 lags one octet ----
        L2 = None
        prev = None
        for oc in range(2, NOCT + 1):
            if oc < NOCT:
                flush_evacs()
                cur = (stage1(oc, L, L2), oc)
                if oc == 10:   # L fully consumed; refill remaining pairs
                    L2 = L_p.tile([4, NL0, W], BF16, name="L")
                    for b in range(8):
                        nc.sync.dma_start(
                            L2[:, 0:NP - NL0, b * 32:(b + 1) * 32],
                            tf[b * 4:(b + 1) * 4, NL0:NP, :])
            else:
                flush_evacs()
                cur = None
            if prev is not None:
                stage2(prev[0], prev[1])
            prev = cur

        for rt in range(2):
            for oh in range(2):
                nc.sync.dma_start(R_d[oh * 128:(oh + 1) * 128, rt],
                                  rsb_all[:, rt, oh])

    nc.compile()
    return nc


_NC_CACHE = {}


def _get_nc():
    if "nc" not in _NC_CACHE:
        _NC_CACHE["nc"] = _build_kernel()
    return _NC_CACHE["nc"]


def _prep_in_maps(S):
    S = np.asarray(S, dtype=np.float32)
    np_bf16 = mybir.dt.np(BF16)
    bw = _build_bw().astype(np_bf16)
    brs = [tuple(x.astype(np_bf16) for x in _build_brp(h)) for h in range(2)]
    Ss = S * np.float32(0.2)
    in_maps = []
    for b in range(B):
        for half in range(2):
            hbase = half * HH
            rows = np.clip(np.arange(hbase - 2, hbase + 130), 0, H - 1)
            shard = Ss[b][:, rows, :].transpose(1, 0, 2)   # [132, K, 256]
            shard = np.ascontiguousarray(shard).astype(np_bf16)
            sm = shard[0:128]
            stail = np.ascontiguousarray(shard[128:132])   # [4, K, 256]
            # fold: SF[wdiv*4 + r', c, j] = ST[r', c, wdiv*32 + j]
            sfold = np.ascontiguousarray(
                stail.reshape(4, K, 8, 32).transpose(2, 0, 1, 3)
                .reshape(32, K, 32))
            in_maps.append({"SM": sm, "ST": stail, "SF": sfold,
                            "BR": brs[half][0], "BR2": brs[half][1],
                            "BW": bw})
    return in_maps


# upper-tri gather index: IU[k, l] = pr for (min,max)
_IU = np.zeros((K, K), dtype=np.int64)
for _k in range(K):
    for _l in range(_k, K):
        _IU[_k, _l] = _IU[_l, _k] = _pr0(_k) + (_l - _k)


def _assemble(results):
    out = np.empty((B, H, W, K, K), dtype=np.float32)
    for i in range(8):
        b, half = divmod(i, 2)
        rd = np.asarray(results[i]["R"]).astype(np.float32)  # [256, 2, 136, 64]
        tri = rd.transpose(1, 3, 0, 2).reshape(HH, W, NP)    # [h, w, pr]
        out[b, half * HH:(half + 1) * HH] = tri[:, :, _IU]
    return out


def kernel(S):
    """S: [4, 16, 256, 256] float32 -> R: [4, 256, 256, 16, 16] float32."""
    nc = _get_nc()
    in_maps = _prep_in_maps(S)
    res = bass_utils.run_bass_kernel_spmd(nc, in_maps, list(range(8)))
    return _assemble(res.results)
